# revision 13
# baseline (speedup 1.0000x reference)
"""Trainium2 Bass kernel for causal self-attention (Wan-style block):

    q = rmsnorm(x @ Wq + bq) * gq ; k = rmsnorm(x @ Wk + bk) * gk
    v = x @ Wv + bv
    rq, rk = rope(q), rope(k)   (interleaved complex pairs)
    out = causal_softmax(rq rk^T / sqrt(hd)) @ v @ Wo + bo

Sharding: tensor-parallel over heads across 8 NeuronCores. Each core owns 2 of
16 heads: it computes its 256 columns of q/k/v (transposed layouts), a tiny
[2, 2048] AllReduce combines the per-core sum-of-squares for the full-width
RMSNorm, each core runs causal attention for its 2 heads and a row-sharded
partial of the output projection; the host sums the 8 partials and adds bo.

All matmuls run in float32r (full PE rate for moving dim >= 256, ~1e-4 rel
err). Data layouts are chosen so no on-device transposes are needed:
  - host feeds x^T; projections compute qT/kT = [col, tok] via W-stationary
    matmuls and v = [tok, col] via xT-stationary matmuls
  - scores are computed directly as S^T = [tk, tq]; softmax denominator via
    ones-vector matmul over partitions; attn^T = [d, tq] via lhsT=v
  - the output partial is produced transposed [D, tok]; host transposes once
"""
import numpy as np

T = 2048
D = 2048
NH = 16
HD = 128
N_CORES = 8
CPD = D // N_CORES      # 256 cols per core
EPS = 1e-6
NKB = 16                # contraction blocks of 128
NCH = 4                 # tq chunks of 512
CH = 512

_module_cache = {}


def _build_module():
    import concourse.bacc as bacc
    import concourse.tile as tile
    from concourse import mybir

    F32 = mybir.dt.float32
    F32R = mybir.dt.float32r
    AF = mybir.ActivationFunctionType

    nc = bacc.Bacc("TRN2", target_bir_lowering=False, debug=False,
                   num_devices=N_CORES)

    # ---- I/O -------------------------------------------------------------
    xT = nc.dram_tensor("xT", [D, T], F32, kind="ExternalInput").ap()
    wq = nc.dram_tensor("wq", [D, CPD], F32, kind="ExternalInput").ap()
    wk = nc.dram_tensor("wk", [D, CPD], F32, kind="ExternalInput").ap()
    wv = nc.dram_tensor("wv", [D, CPD], F32, kind="ExternalInput").ap()
    wo = nc.dram_tensor("wo", [CPD, D], F32, kind="ExternalInput").ap()
    bqk = nc.dram_tensor("bqk", [2, CPD], F32, kind="ExternalInput").ap()
    bv_i = nc.dram_tensor("bv", [1, CPD], F32, kind="ExternalInput").ap()
    g_i = nc.dram_tensor("g", [128, 4], F32, kind="ExternalInput").ap()
    cos_i = nc.dram_tensor("cosT", [128, T], F32, kind="ExternalInput").ap()
    sin_i = nc.dram_tensor("sinT", [128, T], F32, kind="ExternalInput").ap()
    mask_i = nc.dram_tensor("masks", [128, 4 * CH], F32, kind="ExternalInput").ap()
    ones_i = nc.dram_tensor("ones", [128, 2], F32, kind="ExternalInput").ap()

    outT = nc.dram_tensor("outT", [D, T], F32, kind="ExternalOutput").ap()

    swap_mask = [(i ^ 1) for i in range(32)]

    with tile.TileContext(nc) as tc:
        with (
            tc.tile_pool(name="consts", bufs=1) as consts,
            tc.tile_pool(name="qk_sb", bufs=1) as qk_pool,
            tc.tile_pool(name="v_sb", bufs=1) as v_pool,
            tc.tile_pool(name="stats", bufs=1) as stats_pool,
            tc.tile_pool(name="dram", bufs=1, space="DRAM") as dram,
        ):
            # constants
            ones_c = consts.tile([128, 2], F32R, name="ones_c")
            nc.sync.dma_start(ones_c[:], ones_i.bitcast(F32R))
            onesc = ones_c[:, 0:1]            # [128, 1] column of ones
            g_t = consts.tile([128, 4], F32, name="g_t")
            nc.sync.dma_start(g_t[:], g_i)
            bq_t = consts.tile([1, CPD], F32R, name="bq_t")
            bk_t = consts.tile([1, CPD], F32R, name="bk_t")
            nc.sync.dma_start(bq_t[:], bqk[0:1, :].bitcast(F32R))
            nc.sync.dma_start(bk_t[:], bqk[1:2, :].bitcast(F32R))
            b_t = [bq_t, bk_t]
            bv_t = consts.tile([1, CPD], F32R, name="bv_t")
            nc.sync.dma_start(bv_t[:], bv_i.bitcast(F32R))
            # materialized ones rows (broadcast APs are not fed to matmul)
            ones_r = consts.tile([1, CH], F32R, name="ones_r")
            nc.vector.tensor_copy(ones_r[:], ones_c[0:1, 0:1].broadcast_to([1, CH]))
            ones128 = ones_r[0:1, 0:128]      # [1, 128] lhsT for K=1 broadcasts

            # persistent activation storage
            # qk_sb[t][cb]: [128, T] f32r   (q: t=0, k: t=1)
            qk_sb = [[qk_pool.tile([128, T], F32R, name=f"qk{t}{cb}")
                      for cb in range(2)] for t in range(2)]
            v_sb = [v_pool.tile([128, CPD], F32R, name=f"v{tb}")
                    for tb in range(NKB)]
            ssq_sb = stats_pool.tile([1, T], F32, name="ssq_sb")
            ssk_sb = stats_pool.tile([1, T], F32, name="ssk_sb")
            ss_t = [ssq_sb, ssk_sb]
            inv_q = stats_pool.tile([1, T], F32R, name="inv_q")
            inv_k = stats_pool.tile([1, T], F32R, name="inv_k")

            # ---- Phase A1: q/k/v projections -------------------------------
            with (
                tc.tile_pool(name="xt", bufs=9) as xt_pool,
                tc.tile_pool(name="wqk", bufs=1) as w_pool,
                tc.tile_pool(name="wv_p", bufs=1) as wv_pool,
                tc.tile_pool(name="ps_proj", bufs=6, space="PSUM") as ps_proj,
            ):
                w_t = {}
                for t, w_ap in ((0, wq), (1, wk)):
                    for kb in range(NKB):
                        wt = w_pool.tile([128, CPD], F32R, name=f"w{t}_{kb}")
                        nc.sync.dma_start(
                            wt[:], w_ap[kb * 128:(kb + 1) * 128, :].bitcast(F32R))
                        w_t[(t, kb)] = wt
                wv_t = []
                for kb in range(NKB):
                    wt = wv_pool.tile([128, CPD], F32R, name=f"wv_{kb}")
                    nc.sync.dma_start(
                        wt[:], wv[kb * 128:(kb + 1) * 128, :].bitcast(F32R))
                    wv_t.append(wt)

                # process contraction in two halves of 8 kb so that only half
                # of xT is resident at a time (SBUF budget)
                for half in range(2):
                    kbs = list(range(half * 8, half * 8 + 8))
                    xt_t = {}
                    for kb in kbs:
                        xt = xt_pool.tile([128, T], F32R, name=f"xt{kb}",
                                          tag="xt")
                        nc.sync.dma_start(
                            xt[:], xT[kb * 128:(kb + 1) * 128, :].bitcast(F32R))
                        xt_t[kb] = xt

                    # q/k: out[c, t] blocks, W-stationary
                    for t in range(2):
                        for cb in range(2):
                            for ch in range(NCH):
                                pq = ps_proj.tile([128, CH], F32, name="pq",
                                                  tag="pp")
                                for j, kb in enumerate(kbs):
                                    nc.tensor.matmul(
                                        pq[:],
                                        w_t[(t, kb)][:, cb * 128:(cb + 1) * 128],
                                        xt_t[kb][:, ch * CH:(ch + 1) * CH],
                                        start=(j == 0),
                                        stop=(half == 0 and j == 7))
                                if half == 1:
                                    # bias via K=1 matmul closes the group
                                    nc.tensor.matmul(
                                        pq[:],
                                        b_t[t][0:1, cb * 128:(cb + 1) * 128],
                                        ones_r[:],
                                        start=False, stop=True)
                                dst = qk_sb[t][cb][:, ch * CH:(ch + 1) * CH]
                                if half == 0:
                                    nc.vector.tensor_copy(dst, pq[:])
                                else:
                                    nc.vector.tensor_add(dst, dst, pq[:])

                    # v: out[t, c] blocks, xT-stationary
                    for tb in range(NKB):
                        pv = ps_proj.tile([128, CPD], F32, name="pv", tag="pp")
                        for j, kb in enumerate(kbs):
                            nc.tensor.matmul(
                                pv[:],
                                xt_t[kb][:, tb * 128:(tb + 1) * 128],
                                wv_t[kb][:],
                                start=(j == 0), stop=(half == 0 and j == 7))
                        if half == 1:
                            nc.tensor.matmul(
                                pv[:], ones128, bv_t[:],
                                start=False, stop=True)
                        if half == 0:
                            nc.vector.tensor_copy(v_sb[tb][:], pv[:])
                        else:
                            nc.vector.tensor_add(v_sb[tb][:], v_sb[tb][:], pv[:])

            # ---- Phase A2: rms statistics + AllReduce ----------------------
            with (
                tc.tile_pool(name="sq_p", bufs=3) as sq_pool,
                tc.tile_pool(name="ps_ss", bufs=2, space="PSUM") as ps_ss,
            ):
                for t in range(2):
                    for ch in range(NCH):
                        ssp = ps_ss.tile([1, CH], F32, name="ssp", tag="ss")
                        for cb in range(2):
                            sq = sq_pool.tile([128, CH], F32R, name="sq",
                                              tag="sq")
                            nc.scalar.activation(
                                sq[:], qk_sb[t][cb][:, ch * CH:(ch + 1) * CH],
                                AF.Square)
                            nc.tensor.matmul(
                                ssp[:], onesc, sq[:],
                                start=(cb == 0), stop=(cb == 1))
                        nc.vector.tensor_copy(
                            ss_t[t][0:1, ch * CH:(ch + 1) * CH], ssp[:])

                ss_in = dram.tile([2, T], F32, name="ss_in")
                ss_out = dram.tile([2, T], F32, name="ss_out")
                nc.sync.dma_start(ss_in[0:1, :], ssq_sb[:])
                nc.sync.dma_start(ss_in[1:2, :], ssk_sb[:])
                nc.gpsimd.collective_compute(
                    "AllReduce", mybir.AluOpType.add,
                    replica_groups=[list(range(N_CORES))],
                    ins=[ss_in.opt()], outs=[ss_out.opt()],
                )
                nc.sync.dma_start(ssq_sb[:], ss_out[0:1, :])
                nc.sync.dma_start(ssk_sb[:], ss_out[1:2, :])

                # inv_q = 1/sqrt(ss*(HD/D) + HD*eps)   (folds the 1/sqrt(HD)
                # score scale); inv_k = 1/sqrt(ss/D + eps)
                eps_q = sq_pool.tile([1, 1], F32, name="eps_q", tag="eps_q")
                eps_k = sq_pool.tile([1, 1], F32, name="eps_k", tag="eps_k")
                nc.vector.memset(eps_q[:], float(HD * EPS))
                nc.vector.memset(eps_k[:], float(EPS))
                nc.scalar.activation(ssq_sb[:], ssq_sb[:], AF.Sqrt,
                                     bias=eps_q[0:1, 0:1], scale=float(HD / D))
                nc.scalar.activation(ssk_sb[:], ssk_sb[:], AF.Sqrt,
                                     bias=eps_k[0:1, 0:1], scale=float(1.0 / D))
                # f32r out is bit-compatible with f32 here; only the PE
                # interprets the dtype specially
                with nc.allow_low_precision(reason="f32r alias of f32"):
                    nc.vector.reciprocal(inv_q[:], ssq_sb[:])
                    nc.vector.reciprocal(inv_k[:], ssk_sb[:])

            # ---- Phase A3: rope + per-token inverse-rms scaling ------------
            with (
                tc.tile_pool(name="rope_tab", bufs=4) as tab_pool,
                tc.tile_pool(name="rope_scr", bufs=4) as scr_pool,
                tc.tile_pool(name="ps_bc", bufs=2, space="PSUM") as ps_bc,
            ):
                for ch in range(NCH):
                    ct = tab_pool.tile([128, CH], F32R, name="ct", tag="ct")
                    st = tab_pool.tile([128, CH], F32R, name="st", tag="st")
                    nc.sync.dma_start(
                        ct[:], cos_i[:, ch * CH:(ch + 1) * CH].bitcast(F32R))
                    nc.sync.dma_start(
                        st[:], sin_i[:, ch * CH:(ch + 1) * CH].bitcast(F32R))
                    for t in range(2):
                        # broadcast inv over partitions via K=1 matmul
                        inv = inv_q if t == 0 else inv_k
                        ib = ps_bc.tile([128, CH], F32, name="ib", tag="ib")
                        nc.tensor.matmul(
                            ib[:], ones128, inv[:, ch * CH:(ch + 1) * CH],
                            start=True, stop=True)
                        for cb in range(2):
                            sl = qk_sb[t][cb][:, ch * CH:(ch + 1) * CH]
                            qg = scr_pool.tile([128, CH], F32R, name="qg",
                                               tag="qg")
                            sw = scr_pool.tile([128, CH], F32R, name="sw",
                                               tag="sw")
                            nc.vector.tensor_scalar_mul(
                                qg[:], sl, g_t[:, 2 * t + cb:2 * t + cb + 1])
                            nc.vector.stream_shuffle(
                                sw[:].bitcast(F32), qg[:].bitcast(F32),
                                swap_mask)
                            nc.vector.tensor_mul(qg[:], qg[:], ct[:])
                            nc.vector.tensor_mul(sw[:], sw[:], st[:])
                            nc.vector.tensor_add(sl, qg[:], sw[:])
                            nc.vector.tensor_mul(sl, sl, ib[:])

            # ---- Phase B: attention per head -------------------------------
            att_sb = [[None] * NCH for _ in range(2)]
            with tc.tile_pool(name="att", bufs=1) as att_pool:
                with (
                    tc.tile_pool(name="mask_p", bufs=1) as mask_pool,
                    tc.tile_pool(name="est", bufs=6) as est_pool,
                    tc.tile_pool(name="rcp", bufs=4) as rcp_pool,
                    tc.tile_pool(name="ps_st", bufs=2, space="PSUM") as ps_st,
                    tc.tile_pool(name="ps_att", bufs=2, space="PSUM") as ps_att,
                    tc.tile_pool(name="ps_den", bufs=2, space="PSUM") as ps_den,
                    tc.tile_pool(name="ps_rb", bufs=2, space="PSUM") as ps_rb,
                ):
                    mask_t = mask_pool.tile([128, 4 * CH], F32R, name="mask_t")
                    nc.sync.dma_start(mask_t[:], mask_i.bitcast(F32R))

                    for h in range(2):
                        rqh = qk_sb[0][h]
                        rkh = qk_sb[1][h]
                        for ch in range(NCH):
                            nblk = 4 * ch + 4
                            den = ps_den.tile([1, CH], F32, name="den",
                                              tag="den")
                            acc = ps_att.tile([128, CH], F32, name="acc",
                                              tag="acc")
                            for b in range(nblk):
                                stp = ps_st.tile([128, CH], F32, name="stp",
                                                 tag="stp")
                                nc.tensor.matmul(
                                    stp[:], rkh[:, b * 128:(b + 1) * 128],
                                    rqh[:, ch * CH:(ch + 1) * CH],
                                    start=True, stop=True)
                                e = est_pool.tile([128, CH], F32R, name="e",
                                                  tag="e")
                                nc.scalar.activation(e[:], stp[:], AF.Exp)
                                di = b - 4 * ch
                                if di >= 0:
                                    nc.vector.tensor_mul(
                                        e[:], e[:],
                                        mask_t[:, di * CH:(di + 1) * CH])
                                nc.tensor.matmul(den[:], onesc, e[:],
                                                 start=(b == 0),
                                                 stop=(b == nblk - 1))
                                nc.tensor.matmul(
                                    acc[:],
                                    v_sb[b][:, h * 128:(h + 1) * 128], e[:],
                                    start=(b == 0), stop=(b == nblk - 1))
                            rcp = rcp_pool.tile([1, CH], F32R, name="rcp",
                                                tag="rcp")
                            with nc.allow_low_precision(
                                    reason="f32r alias of f32"):
                                nc.vector.reciprocal(rcp[:], den[:])
                            rb = ps_rb.tile([128, CH], F32, name="rb", tag="rb")
                            nc.tensor.matmul(rb[:], ones128, rcp[:],
                                             start=True, stop=True)
                            rbs = rcp_pool.tile([128, CH], F32R, name="rbs",
                                                tag="rbs")
                            nc.vector.tensor_copy(rbs[:], rb[:])
                            at = att_pool.tile([128, CH], F32R,
                                               name=f"att{h}{ch}")
                            nc.vector.tensor_mul(at[:], rbs[:], acc[:])
                            att_sb[h][ch] = at

                # ---- Phase C: output projection (row-sharded partial) ------
                with (
                    tc.tile_pool(name="wo_p", bufs=1) as wo_pool,
                    tc.tile_pool(name="oc_p", bufs=4) as oc_pool,
                    tc.tile_pool(name="ps_o", bufs=4, space="PSUM") as ps_o,
                ):
                    wo_t = []
                    for dh in range(2):
                        wt = wo_pool.tile([128, D], F32R, name=f"wo{dh}")
                        nc.sync.dma_start(
                            wt[:], wo[dh * 128:(dh + 1) * 128, :].bitcast(F32R))
                        wo_t.append(wt)
                    for ch in range(NCH):
                        for dcol in range(NKB):
                            op = ps_o.tile([128, CH], F32, name="op", tag="op")
                            for dh in range(2):
                                nc.tensor.matmul(
                                    op[:],
                                    wo_t[dh][:, dcol * 128:(dcol + 1) * 128],
                                    att_sb[dh][ch][:],
                                    start=(dh == 0), stop=(dh == 1))
                            oc = oc_pool.tile([128, CH], F32, name="oc",
                                              tag="oc")
                            nc.scalar.copy(oc[:], op[:])
                            nc.sync.dma_start(
                                outT[dcol * 128:(dcol + 1) * 128,
                                     ch * CH:(ch + 1) * CH], oc[:])

    nc.compile()
    return nc


def _get_module():
    if "nc" not in _module_cache:
        _module_cache["nc"] = _build_module()
    return _module_cache["nc"]


def _host_prep(x, freqs_cos, freqs_sin, Wq, bq, Wk, bk, Wv, bv, Wo, bo, gq, gk):
    """Build the 8 per-core input maps (all float32 numpy)."""
    f4 = np.float32
    x = np.asarray(x, f4)
    xT = np.ascontiguousarray(x[0].T)
    fc = np.asarray(freqs_cos, f4)
    fs = np.asarray(freqs_sin, f4)

    cosT = np.repeat(fc.T, 2, axis=0)                     # [128, T]
    sinT = np.repeat(fs.T, 2, axis=0)
    sign = np.where(np.arange(128) % 2 == 0, -1.0, 1.0).astype(f4)
    sinT = (sinT * sign[:, None]).astype(f4)

    j = np.arange(CH)[None, :]
    p = np.arange(128)[:, None]
    masks = np.concatenate(
        [(j >= i * 128 + p).astype(f4) for i in range(4)], axis=1)  # [128, 2048]

    ones = np.ones((128, 2), f4)

    Wq = np.asarray(Wq, f4); Wk = np.asarray(Wk, f4)
    Wv = np.asarray(Wv, f4); Wo = np.asarray(Wo, f4)
    bq = np.asarray(bq, f4); bk = np.asarray(bk, f4); bv = np.asarray(bv, f4)
    gq = np.asarray(gq, f4); gk = np.asarray(gk, f4)

    in_maps = []
    for c in range(N_CORES):
        cols = slice(c * CPD, (c + 1) * CPD)
        g_s = np.stack([gq[cols][:128], gq[cols][128:],
                        gk[cols][:128], gk[cols][128:]], axis=1)
        in_maps.append({
            "xT": xT,
            "wq": np.ascontiguousarray(Wq[:, cols]),
            "wk": np.ascontiguousarray(Wk[:, cols]),
            "wv": np.ascontiguousarray(Wv[:, cols]),
            "wo": np.ascontiguousarray(Wo[cols, :]),
            "bqk": np.stack([bq[cols], bk[cols]]),
            "bv": bv[cols][None, :],
            "g": np.ascontiguousarray(g_s),
            "cosT": np.ascontiguousarray(cosT),
            "sinT": np.ascontiguousarray(sinT),
            "masks": np.ascontiguousarray(masks),
            "ones": ones,
        })
    return in_maps


def _run(inputs, trace=False, trace_kwargs=None):
    from concourse import bass_utils
    nc = _get_module()
    in_maps = _host_prep(**inputs)
    res = bass_utils.run_bass_kernel_spmd(
        nc, in_maps, core_ids=list(range(N_CORES)), trace=trace,
        **(trace_kwargs or {}),
    )
    acc = res.results[0]["outT"].astype(np.float64)
    for i in range(1, N_CORES):
        acc += res.results[i]["outT"]
    bo = np.asarray(inputs["bo"], np.float64)
    out = acc.T + bo[None, :]
    return out[None].astype(np.float32), res


def kernel(**inputs):
    out, _ = _run(inputs, trace=False)
    return out


# revision 16
# speedup vs baseline: 1.1864x; 1.1864x over previous
"""Trainium2 Bass kernel for causal self-attention (Wan-style block):

    q = rmsnorm(x @ Wq + bq) * gq ; k = rmsnorm(x @ Wk + bk) * gk
    v = x @ Wv + bv
    rq, rk = rope(q), rope(k)   (interleaved complex pairs)
    out = causal_softmax(rq rk^T / sqrt(hd)) @ v @ Wo + bo

Sharding: tensor-parallel over heads across 8 NeuronCores. Each core owns 2 of
16 heads: it computes its 256 columns of q/k/v (transposed layouts), a tiny
[2, 2048] AllReduce combines the per-core sum-of-squares for the full-width
RMSNorm, each core runs causal attention for its 2 heads and a row-sharded
partial of the output projection; the host sums the 8 partials and adds bo.

Precision: projections and scores run in float32r (full PE rate for moving
dim >= 256, ~1e-4 rel err); the softmax probabilities, attention values and
output projection run in bf16 (error-tolerant, averaged terms). Layouts are
chosen so no on-device transposes are needed:
  - host feeds x^T; projections compute qT/kT = [col, tok] via W-stationary
    matmuls and v = [tok, col] via xT-stationary matmuls
  - scores are computed directly as S^T = [tk, tq]; softmax denominator via
    ones-vector matmul over partitions; attn^T = [d, tq] via lhsT=v
  - the output partial is produced transposed [D, tok] in bf16; the host
    accumulates in f32 and transposes once

Schedule: RMS statistics, their AllReduce, and the rope rotation are
interleaved into the q/k projection stream so the collective and the DVE work
hide behind the v-projection matmuls; the inverse-rms scaling is applied
right after the v pass, then attention runs back-to-back on the PE.
"""
import numpy as np

T = 2048
D = 2048
NH = 16
HD = 128
N_CORES = 8
CPD = D // N_CORES      # 256 cols per core
EPS = 1e-6
NKB = 16                # contraction blocks of 128
NCH = 4                 # tq chunks of 512
CH = 512

_module_cache = {}


def _build_module():
    import concourse.bacc as bacc
    import concourse.tile as tile
    from concourse import mybir

    F32 = mybir.dt.float32
    F32R = mybir.dt.float32r
    BF16 = mybir.dt.bfloat16
    AF = mybir.ActivationFunctionType

    nc = bacc.Bacc("TRN2", target_bir_lowering=False, debug=False,
                   num_devices=N_CORES)

    # ---- I/O -------------------------------------------------------------
    xT = nc.dram_tensor("xT", [D, T], F32, kind="ExternalInput").ap()
    wq = nc.dram_tensor("wq", [D, CPD], F32, kind="ExternalInput").ap()
    wk = nc.dram_tensor("wk", [D, CPD], F32, kind="ExternalInput").ap()
    wv = nc.dram_tensor("wv", [D, CPD], F32, kind="ExternalInput").ap()
    wo = nc.dram_tensor("wo", [CPD, D], BF16, kind="ExternalInput").ap()
    bqk = nc.dram_tensor("bqk", [2, CPD], F32, kind="ExternalInput").ap()
    bv_i = nc.dram_tensor("bv", [1, CPD], F32, kind="ExternalInput").ap()
    g_i = nc.dram_tensor("g", [128, 4], F32, kind="ExternalInput").ap()
    cos_i = nc.dram_tensor("cosT", [128, T], F32, kind="ExternalInput").ap()
    sin_i = nc.dram_tensor("sinT", [128, T], F32, kind="ExternalInput").ap()
    mask_i = nc.dram_tensor("masks", [128, 4 * CH], BF16, kind="ExternalInput").ap()
    ones_i = nc.dram_tensor("ones", [128, 2], F32, kind="ExternalInput").ap()

    outT = nc.dram_tensor("outT", [D, T], BF16, kind="ExternalOutput").ap()

    swap_mask = [(i ^ 1) for i in range(32)]

    with tile.TileContext(nc) as tc:
        with (
            tc.tile_pool(name="consts", bufs=1) as consts,
            tc.tile_pool(name="qk_sb", bufs=1) as qk_pool,
            tc.tile_pool(name="v_bf", bufs=1) as vbf_pool,
            tc.tile_pool(name="stats", bufs=1) as stats_pool,
            tc.tile_pool(name="dram", bufs=1, space="DRAM") as dram,
        ):
            # constants
            ones_c = consts.tile([128, 2], F32R, name="ones_c")
            nc.sync.dma_start(ones_c[:], ones_i.bitcast(F32R))
            onesc = ones_c[:, 0:1]            # [128, 1] ones column (f32r)
            ones_bf = consts.tile([128, 1], BF16, name="ones_bf")
            nc.vector.tensor_copy(ones_bf[:], ones_c[:, 0:1].bitcast(F32))
            g_t = consts.tile([128, 4], F32, name="g_t")
            nc.sync.dma_start(g_t[:], g_i)
            bq_t = consts.tile([1, CPD], F32R, name="bq_t")
            bk_t = consts.tile([1, CPD], F32R, name="bk_t")
            nc.sync.dma_start(bq_t[:], bqk[0:1, :].bitcast(F32R))
            nc.sync.dma_start(bk_t[:], bqk[1:2, :].bitcast(F32R))
            b_t = [bq_t, bk_t]
            bv_t = consts.tile([1, CPD], F32R, name="bv_t")
            nc.sync.dma_start(bv_t[:], bv_i.bitcast(F32R))
            ones_r = consts.tile([1, CH], F32R, name="ones_r")
            nc.vector.tensor_copy(ones_r[:], ones_c[0:1, 0:1].broadcast_to([1, CH]))
            ones128 = ones_r[0:1, 0:128]      # [1, 128] lhsT for K=1 broadcasts

            # persistent activations
            qk_sb = [[qk_pool.tile([128, T], F32R, name=f"qk{t}{cb}")
                      for cb in range(2)] for t in range(2)]
            v_bf = [vbf_pool.tile([128, CPD], BF16, name=f"vb{tb}")
                    for tb in range(NKB)]
            ssq_sb = stats_pool.tile([1, T], F32, name="ssq_sb")
            ssk_sb = stats_pool.tile([1, T], F32, name="ssk_sb")
            ss_t = [ssq_sb, ssk_sb]

            # ---- Phase A: q/k projections + stats + rope -------------------
            with (
                tc.tile_pool(name="xt", bufs=9) as xt_pool,
                tc.tile_pool(name="wqk", bufs=1) as w_pool,
                tc.tile_pool(name="sq_p", bufs=3) as sq_pool,
                tc.tile_pool(name="rope_tab", bufs=2) as tab_pool,
                tc.tile_pool(name="rope_scr", bufs=2) as scr_pool,
                tc.tile_pool(name="ps_proj", bufs=4, space="PSUM") as ps_proj,
                tc.tile_pool(name="ps_ss", bufs=2, space="PSUM") as ps_ss,
            ):
                # xT half 0 first so the first matmuls unblock quickly
                xt_t = {}
                for kb in range(8):
                    t_ = xt_pool.tile([128, T], F32R, name=f"xt{kb}", tag="xt")
                    nc.sync.dma_start(
                        t_[:], xT[kb * 128:(kb + 1) * 128, :].bitcast(F32R))
                    xt_t[kb] = t_
                w_t = {}
                for t, w_ap in ((0, wq), (1, wk)):
                    for kb in range(NKB):
                        wt = w_pool.tile([128, CPD], F32R, name=f"w{t}_{kb}")
                        nc.sync.dma_start(
                            wt[:], w_ap[kb * 128:(kb + 1) * 128, :].bitcast(F32R))
                        w_t[(t, kb)] = wt

                for half in range(2):
                    kbs = list(range(half * 8, half * 8 + 8))
                    if half == 1:
                        for kb in kbs:
                            t_ = xt_pool.tile([128, T], F32R, name=f"xt{kb}",
                                              tag="xt")
                            nc.sync.dma_start(
                                t_[:],
                                xT[kb * 128:(kb + 1) * 128, :].bitcast(F32R))
                            xt_t[kb] = t_

                    for ch in range(NCH):
                        if half == 1:
                            ct = tab_pool.tile([128, CH], F32R, name="ct",
                                               tag="ct")
                            st = tab_pool.tile([128, CH], F32R, name="st",
                                               tag="st")
                            nc.sync.dma_start(
                                ct[:],
                                cos_i[:, ch * CH:(ch + 1) * CH].bitcast(F32R))
                            nc.sync.dma_start(
                                st[:],
                                sin_i[:, ch * CH:(ch + 1) * CH].bitcast(F32R))
                        for t in range(2):
                            ssp = None
                            for cb in range(2):
                                pq = ps_proj.tile([128, CH], F32, name="pq",
                                                  tag="pp")
                                for j, kb in enumerate(kbs):
                                    nc.tensor.matmul(
                                        pq[:],
                                        w_t[(t, kb)][:, cb * 128:(cb + 1) * 128],
                                        xt_t[kb][:, ch * CH:(ch + 1) * CH],
                                        start=(j == 0),
                                        stop=(half == 0 and j == 7))
                                dst = qk_sb[t][cb][:, ch * CH:(ch + 1) * CH]
                                if half == 0:
                                    nc.vector.tensor_copy(dst, pq[:])
                                    continue
                                # close the accumulation with the bias term
                                nc.tensor.matmul(
                                    pq[:],
                                    b_t[t][0:1, cb * 128:(cb + 1) * 128],
                                    ones_r[:],
                                    start=False, stop=True)
                                nc.vector.tensor_add(dst, dst, pq[:])
                                # rms statistics before rope overwrites dst
                                sq = sq_pool.tile([128, CH], F32R, name="sq",
                                                  tag="sq")
                                nc.scalar.activation(sq[:], dst, AF.Square)
                                if cb == 0:
                                    ssp = ps_ss.tile([1, CH], F32, name="ssp",
                                                     tag="ss")
                                nc.tensor.matmul(ssp[:], onesc, sq[:],
                                                 start=(cb == 0),
                                                 stop=(cb == 1))
                                if cb == 1:
                                    nc.vector.tensor_copy(
                                        ss_t[t][0:1, ch * CH:(ch + 1) * CH],
                                        ssp[:])
                                # rope (in place on dst; inv scaling later)
                                qg = scr_pool.tile([128, CH], F32R, name="qg",
                                                   tag="qg")
                                sw = scr_pool.tile([128, CH], F32R, name="sw",
                                                   tag="sw")
                                nc.scalar.activation(
                                    qg[:], dst, AF.Copy,
                                    scale=g_t[:, 2 * t + cb:2 * t + cb + 1])
                                nc.vector.stream_shuffle(
                                    sw[:].bitcast(F32), qg[:].bitcast(F32),
                                    swap_mask)
                                nc.vector.tensor_mul(qg[:], qg[:], ct[:])
                                nc.vector.tensor_mul(sw[:], sw[:], st[:])
                                nc.vector.tensor_add(dst, qg[:], sw[:])

                # stats AllReduce + inverse rms
                ss_in = dram.tile([2, T], F32, name="ss_in")
                ss_out = dram.tile([2, T], F32, name="ss_out")
                nc.sync.dma_start(ss_in[0:1, :], ssq_sb[:])
                nc.sync.dma_start(ss_in[1:2, :], ssk_sb[:])
                nc.gpsimd.collective_compute(
                    "AllReduce", mybir.AluOpType.add,
                    replica_groups=[list(range(N_CORES))],
                    ins=[ss_in.opt()], outs=[ss_out.opt()],
                )
                nc.sync.dma_start(ssq_sb[:], ss_out[0:1, :])
                nc.sync.dma_start(ssk_sb[:], ss_out[1:2, :])


            # ---- Phase A2: v projection (re-streams xT) --------------------
            with (
                tc.tile_pool(name="xt2", bufs=9) as xt2_pool,
                tc.tile_pool(name="wv_p", bufs=1) as wv_pool,
                tc.tile_pool(name="v32", bufs=1) as v32_pool,
                tc.tile_pool(name="inv_p", bufs=1) as inv_pool,
                tc.tile_pool(name="ps_v", bufs=4, space="PSUM") as ps_v,
                tc.tile_pool(name="ps_bc", bufs=2, space="PSUM") as ps_bc,
            ):
                inv_q = inv_pool.tile([1, T], F32R, name="inv_q")
                inv_k = inv_pool.tile([1, T], F32R, name="inv_k")
                v32 = [v32_pool.tile([128, CPD], F32, name=f"v{tb}")
                       for tb in range(NKB)]
                xt2_t = {}
                for kb in range(8):
                    t_ = xt2_pool.tile([128, T], F32R, name=f"x2{kb}", tag="xt2")
                    nc.sync.dma_start(
                        t_[:], xT[kb * 128:(kb + 1) * 128, :].bitcast(F32R))
                    xt2_t[kb] = t_
                wv_t = []
                for kb in range(NKB):
                    wt = wv_pool.tile([128, CPD], F32R, name=f"wv_{kb}")
                    nc.sync.dma_start(
                        wt[:], wv[kb * 128:(kb + 1) * 128, :].bitcast(F32R))
                    wv_t.append(wt)

                for half in range(2):
                    kbs = list(range(half * 8, half * 8 + 8))
                    if half == 1:
                        for kb in kbs:
                            t_ = xt2_pool.tile([128, T], F32R, name=f"x2{kb}",
                                               tag="xt2")
                            nc.sync.dma_start(
                                t_[:],
                                xT[kb * 128:(kb + 1) * 128, :].bitcast(F32R))
                            xt2_t[kb] = t_
                    for tb in range(NKB):
                        pv = ps_v.tile([128, CPD], F32, name="pv", tag="pv")
                        for j, kb in enumerate(kbs):
                            nc.tensor.matmul(
                                pv[:],
                                xt2_t[kb][:, tb * 128:(tb + 1) * 128],
                                wv_t[kb][:],
                                start=(j == 0), stop=(half == 0 and j == 7))
                        if half == 0:
                            nc.vector.tensor_copy(v32[tb][:], pv[:])
                        else:
                            nc.tensor.matmul(
                                pv[:], ones128, bv_t[:],
                                start=False, stop=True)
                            nc.vector.tensor_add(v32[tb][:], v32[tb][:], pv[:])
                            nc.scalar.copy(v_bf[tb][:], v32[tb][:])

                # inv_q = 1/sqrt(ss*(HD/D) + HD*eps)   (folds the 1/sqrt(HD)
                # score scale); inv_k = 1/sqrt(ss/D + eps)
                eps_q = inv_pool.tile([1, 1], F32, name="eps_q")
                eps_k = inv_pool.tile([1, 1], F32, name="eps_k")
                nc.vector.memset(eps_q[:], float(HD * EPS))
                nc.vector.memset(eps_k[:], float(EPS))
                nc.scalar.activation(ssq_sb[:], ssq_sb[:], AF.Sqrt,
                                     bias=eps_q[0:1, 0:1], scale=float(HD / D))
                nc.scalar.activation(ssk_sb[:], ssk_sb[:], AF.Sqrt,
                                     bias=eps_k[0:1, 0:1], scale=float(1.0 / D))
                with nc.allow_low_precision(reason="f32r alias of f32"):
                    nc.vector.reciprocal(inv_q[:], ssq_sb[:])
                    nc.vector.reciprocal(inv_k[:], ssk_sb[:])

                # inverse-rms scaling of rq/rk (AllReduce has long finished)
                for ch in range(NCH):
                    for t in range(2):
                        inv = inv_q if t == 0 else inv_k
                        ib = ps_bc.tile([128, CH], F32, name="ib", tag="ib")
                        nc.tensor.matmul(
                            ib[:], ones128, inv[:, ch * CH:(ch + 1) * CH],
                            start=True, stop=True)
                        for cb in range(2):
                            sl = qk_sb[t][cb][:, ch * CH:(ch + 1) * CH]
                            nc.vector.tensor_mul(sl, sl, ib[:])

            # ---- Phase B: attention per head -------------------------------
            att_sb = [[None] * NCH for _ in range(2)]
            with tc.tile_pool(name="att", bufs=1) as att_pool:
                with (
                    tc.tile_pool(name="mask_p", bufs=1) as mask_pool,
                    tc.tile_pool(name="est", bufs=8) as est_pool,
                    tc.tile_pool(name="rcp", bufs=4) as rcp_pool,
                    tc.tile_pool(name="ps_st", bufs=3, space="PSUM") as ps_st,
                    tc.tile_pool(name="ps_att", bufs=2, space="PSUM") as ps_att,
                    tc.tile_pool(name="ps_den", bufs=2, space="PSUM") as ps_den,
                    tc.tile_pool(name="ps_rb", bufs=1, space="PSUM") as ps_rb,
                ):
                    mask_t = mask_pool.tile([128, 4 * CH], BF16, name="mask_t")
                    nc.sync.dma_start(mask_t[:], mask_i)

                    for h in range(2):
                        rqh = qk_sb[0][h]
                        rkh = qk_sb[1][h]
                        for ch in range(NCH):
                            nblk = 4 * ch + 4
                            den = ps_den.tile([1, CH], F32, name="den",
                                              tag="den")
                            acc = ps_att.tile([128, CH], F32, name="acc",
                                              tag="acc")
                            for b in range(nblk):
                                stp = ps_st.tile([128, CH], F32, name="stp",
                                                 tag="stp")
                                nc.tensor.matmul(
                                    stp[:], rkh[:, b * 128:(b + 1) * 128],
                                    rqh[:, ch * CH:(ch + 1) * CH],
                                    start=True, stop=True)
                                e = est_pool.tile([128, CH], BF16, name="e",
                                                  tag="e")
                                nc.scalar.activation(e[:], stp[:], AF.Exp)
                                di = b - 4 * ch
                                if di >= 0:
                                    nc.vector.tensor_mul(
                                        e[:], e[:],
                                        mask_t[:, di * CH:(di + 1) * CH])
                                nc.tensor.matmul(den[:], ones_bf[:], e[:],
                                                 start=(b == 0),
                                                 stop=(b == nblk - 1))
                                nc.tensor.matmul(
                                    acc[:],
                                    v_bf[b][:, h * 128:(h + 1) * 128], e[:],
                                    start=(b == 0), stop=(b == nblk - 1))
                            rcp = rcp_pool.tile([1, CH], F32R, name="rcp",
                                                tag="rcp")
                            with nc.allow_low_precision(
                                    reason="f32r alias of f32"):
                                nc.vector.reciprocal(rcp[:], den[:])
                            rb = ps_rb.tile([128, CH], F32, name="rb", tag="rb")
                            nc.tensor.matmul(rb[:], ones128, rcp[:],
                                             start=True, stop=True)
                            rbs = rcp_pool.tile([128, CH], F32, name="rbs",
                                                tag="rbs")
                            nc.vector.tensor_copy(rbs[:], rb[:])
                            at = att_pool.tile([128, CH], BF16,
                                               name=f"att{h}{ch}")
                            nc.vector.tensor_mul(at[:], rbs[:], acc[:])
                            att_sb[h][ch] = at

                # ---- Phase C: output projection (row-sharded partial) ------
                with (
                    tc.tile_pool(name="wo_p", bufs=1) as wo_pool,
                    tc.tile_pool(name="oc_p", bufs=4) as oc_pool,
                    tc.tile_pool(name="ps_o", bufs=4, space="PSUM") as ps_o,
                ):
                    wo_t = []
                    for dh in range(2):
                        wt = wo_pool.tile([128, D], BF16, name=f"wo{dh}")
                        nc.sync.dma_start(wt[:], wo[dh * 128:(dh + 1) * 128, :])
                        wo_t.append(wt)
                    for ch in range(NCH):
                        for dcol in range(NKB):
                            op = ps_o.tile([128, CH], F32, name="op", tag="op")
                            for dh in range(2):
                                nc.tensor.matmul(
                                    op[:],
                                    wo_t[dh][:, dcol * 128:(dcol + 1) * 128],
                                    att_sb[dh][ch][:],
                                    start=(dh == 0), stop=(dh == 1))
                            oc = oc_pool.tile([128, CH], BF16, name="oc",
                                              tag="oc")
                            nc.vector.tensor_copy(oc[:], op[:])
                            nc.sync.dma_start(
                                outT[dcol * 128:(dcol + 1) * 128,
                                     ch * CH:(ch + 1) * CH], oc[:])

    nc.compile()
    return nc


def _get_module():
    if "nc" not in _module_cache:
        _module_cache["nc"] = _build_module()
    return _module_cache["nc"]


def _host_prep(x, freqs_cos, freqs_sin, Wq, bq, Wk, bk, Wv, bv, Wo, bo, gq, gk):
    """Build the 8 per-core input maps."""
    import ml_dtypes
    f4 = np.float32
    bf = ml_dtypes.bfloat16
    x = np.asarray(x, f4)
    xT = np.ascontiguousarray(x[0].T)
    fc = np.asarray(freqs_cos, f4)
    fs = np.asarray(freqs_sin, f4)

    cosT = np.repeat(fc.T, 2, axis=0)                     # [128, T]
    sinT = np.repeat(fs.T, 2, axis=0)
    sign = np.where(np.arange(128) % 2 == 0, -1.0, 1.0).astype(f4)
    sinT = (sinT * sign[:, None]).astype(f4)

    j = np.arange(CH)[None, :]
    p = np.arange(128)[:, None]
    masks = np.concatenate(
        [(j >= i * 128 + p).astype(f4) for i in range(4)], axis=1).astype(bf)

    ones = np.ones((128, 2), f4)

    Wq = np.asarray(Wq, f4); Wk = np.asarray(Wk, f4)
    Wv = np.asarray(Wv, f4); Wo = np.asarray(Wo, f4)
    bq = np.asarray(bq, f4); bk = np.asarray(bk, f4); bv = np.asarray(bv, f4)
    gq = np.asarray(gq, f4); gk = np.asarray(gk, f4)

    in_maps = []
    for c in range(N_CORES):
        cols = slice(c * CPD, (c + 1) * CPD)
        g_s = np.stack([gq[cols][:128], gq[cols][128:],
                        gk[cols][:128], gk[cols][128:]], axis=1)
        in_maps.append({
            "xT": xT,
            "wq": np.ascontiguousarray(Wq[:, cols]),
            "wk": np.ascontiguousarray(Wk[:, cols]),
            "wv": np.ascontiguousarray(Wv[:, cols]),
            "wo": np.ascontiguousarray(Wo[cols, :]).astype(bf),
            "bqk": np.stack([bq[cols], bk[cols]]),
            "bv": bv[cols][None, :],
            "g": np.ascontiguousarray(g_s),
            "cosT": np.ascontiguousarray(cosT),
            "sinT": np.ascontiguousarray(sinT),
            "masks": np.ascontiguousarray(masks),
            "ones": ones,
        })
    return in_maps


def _run(inputs, trace=False, trace_kwargs=None):
    from concourse import bass_utils
    nc = _get_module()
    in_maps = _host_prep(**inputs)
    res = bass_utils.run_bass_kernel_spmd(
        nc, in_maps, core_ids=list(range(N_CORES)), trace=trace,
        **(trace_kwargs or {}),
    )
    acc = res.results[0]["outT"].astype(np.float64)
    for i in range(1, N_CORES):
        acc += res.results[i]["outT"].astype(np.float64)
    bo = np.asarray(inputs["bo"], np.float64)
    out = acc.T + bo[None, :]
    return out[None].astype(np.float32), res


def kernel(**inputs):
    out, _ = _run(inputs, trace=False)
    return out
